# revision 1
# baseline (speedup 1.0000x reference)
"""Bidirectional cross-attention Trainium2 kernel (8 NeuronCores, SPMD).

Sharding: 2 heads per core (16 heads / 8 cores); both batches on every core.
Host pre-transposes x/context to [D, B*N] bf16, slices per-head weight columns.
Device: projections -> two symmetric attention passes (row softmax of sim and
of sim^T, via unnormalized exp + fused denominator row) -> AllToAll to
sequence-shard the merged heads -> output projections -> per-core slices.
"""

import numpy as np
import ml_dtypes

BF16 = ml_dtypes.bfloat16

# problem constants (hardcoded per contract)
B = 2
N = 2048
D = 1024
HEADS = 16
DIM_HEAD = 64
SCALE = DIM_HEAD ** -0.5

N_CORES = 8
HD = 128            # per-core head dims (2 heads x 64)
A = B * N           # 4096 flattened tokens
SL = N // N_CORES   # 256 per-batch output slice per core
KC = D // 128       # 8 contraction chunks for projections
AC_W = 512          # projection output chunk width
N_AT = 2048 // 128  # 16 partition tiles per batch in attention


def build_nc(reps=1, use_cc=True):
    import concourse.bacc as bacc
    import concourse.tile as tile
    from concourse import mybir
    from concourse.masks import make_identity

    fp32 = mybir.dt.float32
    bf16 = mybir.dt.bfloat16
    EXP = mybir.ActivationFunctionType.Exp

    nc = bacc.Bacc("TRN2", target_bir_lowering=False, num_devices=N_CORES)

    # ---- I/O ----
    xT = nc.dram_tensor("xT", [D, A], bf16, kind="ExternalInput")
    cT = nc.dram_tensor("cT", [D, A], bf16, kind="ExternalInput")
    wqk = nc.dram_tensor("wqk", [D, HD], bf16, kind="ExternalInput")
    wv = nc.dram_tensor("wv", [D, HD], bf16, kind="ExternalInput")
    wcqk = nc.dram_tensor("wcqk", [D, HD], bf16, kind="ExternalInput")
    wcv = nc.dram_tensor("wcv", [D, HD], bf16, kind="ExternalInput")
    wout = nc.dram_tensor("wout", [D, D], bf16, kind="ExternalInput")
    wcout = nc.dram_tensor("wcout", [D, D], bf16, kind="ExternalInput")
    out_sl = nc.dram_tensor("out_sl", [2 * SL, D], fp32, kind="ExternalOutput")
    ctx_sl = nc.dram_tensor("ctx_sl", [2 * SL, D], fp32, kind="ExternalOutput")

    # collective bounce buffers, one per direction x batch (shard-major)
    cc_in_o0 = nc.dram_tensor("cc_in_o0", [N_CORES * HD, SL], bf16)
    cc_in_o1 = nc.dram_tensor("cc_in_o1", [N_CORES * HD, SL], bf16)
    cc_out_o0 = nc.dram_tensor("cc_out_o0", [N_CORES * HD, SL], bf16)
    cc_out_o1 = nc.dram_tensor("cc_out_o1", [N_CORES * HD, SL], bf16)
    cc_in_c0 = nc.dram_tensor("cc_in_c0", [N_CORES * HD, SL], bf16)
    cc_in_c1 = nc.dram_tensor("cc_in_c1", [N_CORES * HD, SL], bf16)
    cc_out_c0 = nc.dram_tensor("cc_out_c0", [N_CORES * HD, SL], bf16)
    cc_out_c1 = nc.dram_tensor("cc_out_c1", [N_CORES * HD, SL], bf16)

    with tile.TileContext(nc) as tc:
        with tc.tile_pool(name="singles", bufs=1) as singles:
            # ---- long-lived SBUF tensors ----
            wqk_sb = singles.tile([128, KC, HD], bf16)
            wv_sb = singles.tile([128, KC, HD], bf16)
            wcqk_sb = singles.tile([128, KC, HD], bf16)
            wcv_sb = singles.tile([128, KC, HD], bf16)
            for w_dram, w_sb in ((wqk, wqk_sb), (wv, wv_sb), (wcqk, wcqk_sb), (wcv, wcv_sb)):
                wv_ = w_dram.ap().rearrange("(k p) m -> p k m", p=128)
                nc.sync.dma_start(out=w_sb[:, 0:1, :], in_=wv_[:, 0:1, :])
                nc.sync.dma_start(out=w_sb[:, 1:KC, :], in_=wv_[:, 1:KC, :])

            wout_sb = singles.tile([128, KC, D], bf16)
            wcout_sb = singles.tile([128, KC, D], bf16)

            qkT_sb = singles.tile([128, A], bf16)     # [hd, a]
            cqkT_sb = singles.tile([128, A], bf16)    # [hd, c]
            # per-head natural-layout values with fused ones column: blocks of 65
            v0_sb = singles.tile([128, A // 128, 65], bf16)
            v1_sb = singles.tile([128, A // 128, 65], bf16)
            cv0_sb = singles.tile([128, A // 128, 65], bf16)
            cv1_sb = singles.tile([128, A // 128, 65], bf16)
            for t in (v0_sb, v1_sb, cv0_sb, cv1_sb):
                nc.vector.memset(t, 1.0)

            ident = singles.tile([128, 128], bf16)
            make_identity(nc, ident)

            ones_col = singles.tile([65, 64], bf16)
            nc.vector.memset(ones_col, 1.0)

            # per-head unnormalized attention outputs [d, pos]
            oa_h0 = singles.tile([64, A], bf16)   # dir1: out
            oa_h1 = singles.tile([64, A], bf16)
            ca_h0 = singles.tile([64, A], bf16)   # dir2: ctx_out
            ca_h1 = singles.tile([64, A], bf16)
            oa_h = [oa_h0, oa_h1]
            ca_h = [ca_h0, ca_h1]

            for _rep in range(reps):
                # ================= P1: projections =================
                with (
                    tc.tile_pool(name="p1sb", bufs=3) as p1sb,
                    tc.tile_pool(name="p1scr", bufs=1) as p1scr,
                ):
                    vT_sb = p1scr.tile([128, A], bf16)    # [hd, a] pre-transpose scratch
                    cvT_sb = p1scr.tile([128, A], bf16)
                    n_ac = A // AC_W
                    ps1cm = tc.tile_pool(name="ps1", bufs=8, space="PSUM")
                    ps1 = ps1cm.__enter__()
                    for ac in range(n_ac):
                        x_t = p1sb.tile([128, KC, AC_W], bf16, tag="xin")
                        c_t = p1sb.tile([128, KC, AC_W], bf16, tag="cin")
                        nsplit = KC if ac == 0 else 2
                        step = KC // nsplit
                        for si in range(nsplit):
                            ks = slice(si * step, (si + 1) * step)
                            nc.sync.dma_start(
                                out=x_t[:, ks, :],
                                in_=xT.ap().rearrange("(k p) a -> p k a", p=128)[:, ks, ac * AC_W:(ac + 1) * AC_W],
                            )
                            nc.sync.dma_start(
                                out=c_t[:, ks, :],
                                in_=cT.ap().rearrange("(k p) a -> p k a", p=128)[:, ks, ac * AC_W:(ac + 1) * AC_W],
                            )
                        qk_ps = ps1.tile([128, AC_W], fp32, tag="p1")
                        v_ps = ps1.tile([128, AC_W], fp32, tag="p1")
                        cqk_ps = ps1.tile([128, AC_W], fp32, tag="p1")
                        cv_ps = ps1.tile([128, AC_W], fp32, tag="p1")
                        for kc in range(KC):
                            st = kc == 0
                            sp = kc == KC - 1
                            nc.tensor.matmul(qk_ps, wqk_sb[:, kc, :], x_t[:, kc, :], start=st, stop=sp)
                            nc.tensor.matmul(v_ps, wv_sb[:, kc, :], x_t[:, kc, :], start=st, stop=sp)
                            nc.tensor.matmul(cqk_ps, wcqk_sb[:, kc, :], c_t[:, kc, :], start=st, stop=sp)
                            nc.tensor.matmul(cv_ps, wcv_sb[:, kc, :], c_t[:, kc, :], start=st, stop=sp)
                        sl_ = slice(ac * AC_W, (ac + 1) * AC_W)
                        nc.vector.tensor_copy(qkT_sb[:, sl_], qk_ps)
                        nc.vector.tensor_copy(cqkT_sb[:, sl_], cqk_ps)
                        nc.scalar.copy(vT_sb[:, sl_], v_ps)
                        nc.scalar.copy(cvT_sb[:, sl_], cv_ps)

                    ps1cm.__exit__(None, None, None)
                    # ---- P1b: transpose v/cv to natural layout, split heads ----
                    ps1tcm = tc.tile_pool(name="ps1t", bufs=4, space="PSUM")
                    ps1t = ps1tcm.__enter__()
                    for src_sb, d0, d1 in ((vT_sb, v0_sb, v1_sb), (cvT_sb, cv0_sb, cv1_sb)):
                        for ac in range(n_ac):
                            tr_ps = ps1t.tile([128, AC_W], bf16, tag="tr")
                            for i in range(AC_W // 128):
                                blk = ac * AC_W + i * 128
                                nc.tensor.transpose(
                                    tr_ps[:, i * 128:(i + 1) * 128],
                                    src_sb[:, blk:blk + 128],
                                    ident,
                                )
                            trv = tr_ps.rearrange("p (i m) -> p i m", m=128)
                            nb = AC_W // 128
                            a0 = ac * nb
                            nc.vector.tensor_copy(d0[:, a0:a0 + nb, 0:64], trv[:, :, 0:64])
                            nc.vector.tensor_copy(d1[:, a0:a0 + nb, 0:64], trv[:, :, 64:128])

                    ps1tcm.__exit__(None, None, None)
                # ================= P2: attention =================
                with (
                    tc.tile_pool(name="e_pool", bufs=8) as e_pool,
                    tc.tile_pool(name="norm", bufs=3) as norm_pool,
                    tc.tile_pool(name="ps2", bufs=2, space="PSUM") as ps2,
                ):
                    def attn_pass(b, statT, movT, vo0, vo1, dst0, dst1):
                        # sim tiles [stat-pos(128), mov-pos], exp, PV with fused
                        # denominator row, normalize, write dst [64, mov-pos].
                        base = b * 2048
                        for cw in range(2):  # 1024-wide mov window
                            wbase = base + cw * 1024
                            pv0 = ps2.tile([65, 1024], fp32, tag="pv")
                            pv1 = ps2.tile([65, 1024], fp32, tag="pv")
                            for at in range(N_AT):
                                a0 = base + at * 128
                                for cs in range(2):  # 512-wide sub-window
                                    m0 = wbase + cs * 512
                                    s = ps2.tile([128, 1024], fp32, tag="sim")
                                    nc.tensor.matmul(
                                        s[:, 0:512],
                                        statT[0:64, a0:a0 + 128],
                                        movT[0:64, m0:m0 + 512],
                                        start=True, stop=True,
                                    )
                                    nc.tensor.matmul(
                                        s[:, 512:1024],
                                        statT[64:128, a0:a0 + 128],
                                        movT[64:128, m0:m0 + 512],
                                        start=True, stop=True,
                                        tile_position=(64, 0),
                                    )
                                    e = e_pool.tile([128, 1024], bf16, tag="e")
                                    nc.scalar.activation(e, s, EXP, scale=SCALE)
                                    gat = b * N_AT + at
                                    st = at == 0
                                    sp = at == N_AT - 1
                                    nc.tensor.matmul(
                                        pv0[:, cs * 512:(cs + 1) * 512],
                                        vo0[:, gat, :], e[:, 0:512],
                                        start=st, stop=sp, skip_group_check=True,
                                    )
                                    nc.tensor.matmul(
                                        pv1[:, cs * 512:(cs + 1) * 512],
                                        vo1[:, gat, :], e[:, 512:1024],
                                        start=st, stop=sp, skip_group_check=True,
                                    )
                            # normalize: dst = pv[0:64] * bcast(1 / pv[64]).
                            # pv is copied to SBUF first; the dead pv PSUM rows
                            # then host the broadcast so no sim slot is taken.
                            for pv, dst in ((pv0, dst0), (pv1, dst1)):
                                pvsb = norm_pool.tile([65, 1024], fp32, tag="pvsb")
                                nc.vector.tensor_copy(pvsb, pv)
                                nc.vector.reciprocal(pvsb[64:65, :], pvsb[64:65, :])
                                recb = norm_pool.tile([65, 1024], bf16, tag="recb")
                                nc.vector.tensor_copy(recb[64:65, :], pvsb[64:65, :])
                                for g in range(2):
                                    nc.tensor.matmul(
                                        pv[0:64, g * 512:(g + 1) * 512],
                                        ones_col[64:65, :],
                                        recb[64:65, g * 512:(g + 1) * 512],
                                        start=True, stop=True, skip_group_check=True,
                                    )
                                nc.vector.tensor_mul(
                                    dst[:, wbase:wbase + 1024], pvsb[0:64, :], pv[0:64, :]
                                )

                    def bounce_and_a2a(srcs, b, cc_in, cc_out):
                        ccv = cc_in.ap().rearrange("(r p) j -> r p j", p=HD)
                        for r in range(N_CORES):
                            for h, src_t in enumerate(srcs):
                                nc.sync.dma_start(
                                    out=ccv[r, h * 64:(h + 1) * 64],
                                    in_=src_t[:, b * 2048 + r * SL:b * 2048 + (r + 1) * SL],
                                )
                        if use_cc:
                            nc.gpsimd.collective_compute(
                                "AllToAll", mybir.AluOpType.bypass,
                                replica_groups=[list(range(N_CORES))],
                                ins=[cc_in.ap().opt()], outs=[cc_out.ap().opt()],
                            )
                        else:
                            nc.sync.dma_start(out=cc_out.ap(), in_=cc_in.ap())

                    # pass A (dir2: context_out):  stat=qk(a), mov=cqk(c), values=v
                    attn_pass(0, qkT_sb, cqkT_sb, v0_sb, v1_sb, ca_h[0], ca_h[1])
                    bounce_and_a2a(ca_h, 0, cc_in_c0, cc_out_c0)
                    attn_pass(1, qkT_sb, cqkT_sb, v0_sb, v1_sb, ca_h[0], ca_h[1])
                    bounce_and_a2a(ca_h, 1, cc_in_c1, cc_out_c1)
                    # pass B (dir1: out):  stat=cqk(c), mov=qk(a), values=cv
                    attn_pass(0, cqkT_sb, qkT_sb, cv0_sb, cv1_sb, oa_h[0], oa_h[1])
                    bounce_and_a2a(oa_h, 0, cc_in_o0, cc_out_o0)
                    nc.sync.dma_start(out=wcout_sb, in_=wcout.ap().rearrange("(k p) g -> p k g", p=128))
                    nc.sync.dma_start(out=wout_sb, in_=wout.ap().rearrange("(k p) g -> p k g", p=128))
                    attn_pass(1, cqkT_sb, qkT_sb, cv0_sb, cv1_sb, oa_h[0], oa_h[1])

                    # ============ P4: output projections (inside P2 pools) ============
                    with (
                        tc.tile_pool(name="s3in", bufs=2) as s3in,
                        tc.tile_pool(name="s3out", bufs=2) as s3out,
                    ):
                        def stage3(full, w_sb, dst, at2_range):
                            for at2 in at2_range:
                                ps3 = ps2.tile([128, D], fp32, tag="sim")
                                for kc in range(KC):
                                    for g in range(2):
                                        nc.tensor.matmul(
                                            ps3[:, g * 512:(g + 1) * 512],
                                            full[:, kc, at2 * 128:(at2 + 1) * 128],
                                            w_sb[:, kc, g * 512:(g + 1) * 512],
                                            start=(kc == 0), stop=(kc == KC - 1),
                                            skip_group_check=True,
                                        )
                                o_sb = s3out.tile([128, D], fp32, tag="o")
                                nc.vector.tensor_copy(o_sb, ps3)
                                nc.sync.dma_start(
                                    out=dst.ap()[at2 * 128:(at2 + 1) * 128, :], in_=o_sb
                                )

                        # issue everything that does NOT depend on the last A2A
                        # (ctx both halves, oa b0 half) BEFORE the last collective,
                        # so PE fills the collective's latency and no DMA-lane
                        # false-serialization gates it.
                        full_c = s3in.tile([128, KC, 2 * SL], bf16, tag="full")
                        for bi, cc in enumerate((cc_out_c0, cc_out_c1)):
                            nc.sync.dma_start(
                                out=full_c[:, :, bi * SL:(bi + 1) * SL],
                                in_=cc.ap().rearrange("(k p) a -> p k a", p=128),
                            )
                        # bounce DMAs for oa-b1 (collective issued after stage3 work)
                        ccv = cc_in_o1.ap().rearrange("(r p) j -> r p j", p=HD)
                        for r in range(N_CORES):
                            for h, src_t in enumerate(oa_h):
                                nc.sync.dma_start(
                                    out=ccv[r, h * 64:(h + 1) * 64],
                                    in_=src_t[:, 2048 + r * SL:2048 + (r + 1) * SL],
                                )
                        full_o = s3in.tile([128, KC, 2 * SL], bf16, tag="full")
                        nc.gpsimd.dma_start(
                            out=full_o[:, :, 0:SL],
                            in_=cc_out_o0.ap().rearrange("(k p) a -> p k a", p=128),
                        )
                        if use_cc:
                            nc.gpsimd.collective_compute(
                                "AllToAll", mybir.AluOpType.bypass,
                                replica_groups=[list(range(N_CORES))],
                                ins=[cc_in_o1.ap().opt()], outs=[cc_out_o1.ap().opt()],
                            )
                        else:
                            nc.gpsimd.dma_start(out=cc_out_o1.ap(), in_=cc_in_o1.ap())
                        stage3(full_c, wcout_sb, ctx_sl, range(4))
                        stage3(full_o, wout_sb, out_sl, range(2))
                        nc.gpsimd.dma_start(
                            out=full_o[:, :, SL:2 * SL],
                            in_=cc_out_o1.ap().rearrange("(k p) a -> p k a", p=128),
                        )
                        stage3(full_o, wout_sb, out_sl, range(2, 4))
    nc.compile()
    return nc


_NC_CACHE = {}


def _get_nc():
    if "nc" not in _NC_CACHE:
        _NC_CACHE["nc"] = build_nc()
    return _NC_CACHE["nc"]


def _run(in_maps, trace=False):
    from concourse.bass_utils import run_bass_kernel_spmd
    nc = _get_nc()
    return run_bass_kernel_spmd(nc, in_maps, core_ids=list(range(N_CORES)), trace=trace)


def prepare_in_maps(x, context, w_qk, w_v, w_cqk, w_cv, w_out, w_cout):
    x = np.asarray(x, dtype=np.float32)
    context = np.asarray(context, dtype=np.float32)
    xT = np.ascontiguousarray(x.reshape(A, D).T).astype(BF16)
    cT = np.ascontiguousarray(context.reshape(A, D).T).astype(BF16)
    wout_b = np.asarray(w_out, np.float32).astype(BF16)
    wcout_b = np.asarray(w_cout, np.float32).astype(BF16)
    in_maps = []
    for c in range(N_CORES):
        cs = slice(c * HD, (c + 1) * HD)
        in_maps.append({
            "xT": xT,
            "cT": cT,
            "wqk": np.ascontiguousarray(np.asarray(w_qk, np.float32)[:, cs]).astype(BF16),
            "wv": np.ascontiguousarray(np.asarray(w_v, np.float32)[:, cs]).astype(BF16),
            "wcqk": np.ascontiguousarray(np.asarray(w_cqk, np.float32)[:, cs]).astype(BF16),
            "wcv": np.ascontiguousarray(np.asarray(w_cv, np.float32)[:, cs]).astype(BF16),
            "wout": wout_b,
            "wcout": wcout_b,
        })
    return in_maps


def assemble(results):
    out = np.empty((B, N, D), np.float32)
    ctx = np.empty((B, N, D), np.float32)
    for c in range(N_CORES):
        o = results[c]["out_sl"].reshape(B, SL, D)
        k = results[c]["ctx_sl"].reshape(B, SL, D)
        out[:, c * SL:(c + 1) * SL, :] = o
        ctx[:, c * SL:(c + 1) * SL, :] = k
    return out, ctx


def kernel(x, context, w_qk, w_v, w_cqk, w_cv, w_out, w_cout):
    in_maps = prepare_in_maps(x, context, w_qk, w_v, w_cqk, w_cv, w_out, w_cout)
    res = _run(in_maps)
    return assemble(res.results)



# revision 18
# speedup vs baseline: 1.2222x; 1.2222x over previous
"""Bidirectional cross-attention Trainium2 kernel (8 NeuronCores, SPMD).

Sharding: 2 heads per core (16 heads / 8 cores); both batches on every core.
Host pre-transposes x/context to [D, B*N] bf16, slices per-head weight columns.

Per batch the kernel runs two symmetric attention layouts:
  L1: sim[i,j] tiles (stat=qk, mov=cqk) -> exp -> E; PV-ctx uses E as the
      STATIONARY operand with [v|1] moving (free dim 65), producing
      ctx_out in token-partition layout where softmax normalization is a
      native per-partition reciprocal + tensor_scalar multiply.
  L2: sim^T[j,i] tiles (stat=cqk, mov=qk) -> exp -> E^T; PV-out symmetric.
Projection work (P1) is emitted piece-wise between attention iterations so
the scalar engine never starves. Normalized outputs are staged [tok, d]
major, bounced progressively per window, exchanged with an AllToAll per
(batch, direction) - the last exchange split in two half-collectives so it
pipelines with the end of compute - transposed back to [d, tok] after the
exchange, projected (stage3) and written as per-core token slices.
"""

import numpy as np
import ml_dtypes

BF16 = ml_dtypes.bfloat16

# problem constants (hardcoded per contract)
B = 2
N = 2048
D = 1024
HEADS = 16
DIM_HEAD = 64
SCALE = DIM_HEAD ** -0.5

N_CORES = 8
HD = 128            # per-core head dims (2 heads x 64)
A = B * N           # 4096 flattened tokens
SL = N // N_CORES   # 256 per-batch output slice per core
KC = D // 128       # 8 contraction chunks for projections
NT = N // 128       # 16 token tiles per batch


def build_nc(reps=1, use_cc=True):
    import concourse.bacc as bacc
    import concourse.tile as tile
    from concourse import mybir
    from concourse.masks import make_identity

    fp32 = mybir.dt.float32
    bf16 = mybir.dt.bfloat16
    EXP = mybir.ActivationFunctionType.Exp

    nc = bacc.Bacc("TRN2", target_bir_lowering=False, num_devices=N_CORES)

    # ---- I/O ----
    xT = nc.dram_tensor("xT", [D, A], bf16, kind="ExternalInput")
    cT = nc.dram_tensor("cT", [D, A], bf16, kind="ExternalInput")
    wqk = nc.dram_tensor("wqk", [D, HD], bf16, kind="ExternalInput")
    wv = nc.dram_tensor("wv", [D, HD], bf16, kind="ExternalInput")
    wcqk = nc.dram_tensor("wcqk", [D, HD], bf16, kind="ExternalInput")
    wcv = nc.dram_tensor("wcv", [D, HD], bf16, kind="ExternalInput")
    wout = nc.dram_tensor("wout", [D, D], bf16, kind="ExternalInput")
    wcout = nc.dram_tensor("wcout", [D, D], bf16, kind="ExternalInput")
    out_sl = nc.dram_tensor("out_sl", [2 * SL, D], fp32, kind="ExternalOutput")
    ctx_sl = nc.dram_tensor("ctx_sl", [2 * SL, D], fp32, kind="ExternalOutput")

    # collective bounce buffers; rows are token-major (= core-slice-major).
    # The final exchange (out dir, batch 1) is split into two half buffers
    # with rows (core, token-within-half) so each half is a valid AllToAll.
    cc_bufs = {}
    for dname in ("c", "o"):
        for b in range(B):
            for hh in range(2):
                cc_bufs[(dname, b, hh, "in")] = nc.dram_tensor(f"cc_in_{dname}{b}{hh}", [N // 2, HD], bf16)
                cc_bufs[(dname, b, hh, "out")] = nc.dram_tensor(f"cc_out_{dname}{b}{hh}", [N // 2, HD], bf16)

    with tile.TileContext(nc) as tc:
        with (
            tc.tile_pool(name="singles", bufs=1) as singles,
            tc.tile_pool(name="xc", bufs=4) as xc_pool,
            tc.tile_pool(name="epool", bufs=5) as e_pool,
            tc.tile_pool(name="rec", bufs=4) as rec_pool,
            tc.tile_pool(name="osb", bufs=2) as osb_pool,
            tc.tile_pool(name="ldp", bufs=3) as ld_pool,
            tc.tile_pool(name="ps_sim", bufs=3, space="PSUM") as ps_sim,
            tc.tile_pool(name="ps_pv", bufs=1, space="PSUM") as ps_pv,
        ):
            # ---- long-lived SBUF tensors ----
            wqk_sb = singles.tile([128, KC, HD], bf16)
            wv_sb = singles.tile([128, KC, HD], bf16)
            wcqk_sb = singles.tile([128, KC, HD], bf16)
            wcv_sb = singles.tile([128, KC, HD], bf16)
            def load_w(w_dram, w_sb):
                wv_ = w_dram.ap().rearrange("(k p) m -> p k m", p=128)
                nc.sync.dma_start(out=w_sb[:, 0:4, :], in_=wv_[:, 0:4, :])
                nc.sync.dma_start(out=w_sb[:, 4:KC, :], in_=wv_[:, 4:KC, :])
            load_w(wcqk, wcqk_sb)
            load_w(wqk, wqk_sb)

            wout_sb = singles.tile([128, KC, D], bf16)
            wcout_sb = singles.tile([128, KC, D], bf16)

            qkT_sb = singles.tile([128, A], bf16)     # [2h*64d, a]
            cqkT_sb = singles.tile([128, A], bf16)    # [2h*64d, c]
            # per-head natural-layout values with trailing ones column
            v_nat = [singles.tile([128, A // 128, 65], bf16, name=f"v_nat{h}", tag=f"v_nat{h}") for h in range(2)]
            cv_nat = [singles.tile([128, A // 128, 65], bf16, name=f"cv_nat{h}", tag=f"cv_nat{h}") for h in range(2)]
            for t in v_nat + cv_nat:
                nc.vector.memset(t[:, :, 64:65], 1.0)

            ident = singles.tile([128, 128], bf16)
            make_identity(nc, ident)
            # warm-up chain: ramp the PE p-state before the first projection
            warm = ps_sim.tile([128, 128], fp32, tag="sim", name="warm")
            for wi in range(30):
                nc.tensor.matmul(warm, ident, ident,
                                 start=(wi == 0), stop=(wi == 29),
                                 skip_group_check=True)

            # normalized attention outputs, token-partition staging
            # [tok mod 128, tok tile 16, head 2, d 64]
            stg = {
                (dname, b): singles.tile([128, NT, 2, 64], bf16, name=f"stg_{dname}{b}", tag=f"stg_{dname}{b}")
                for dname in ("c", "o") for b in range(B)
            }
            # post-A2A [d, tok] tensors for stage3: [128 d, KC(=src core), 2*SL]
            full = {"c": singles.tile([128, KC, 2 * SL], bf16, name="full_c", tag="full_c"),
                    "o": singles.tile([128, KC, 2 * SL], bf16, name="full_o", tag="full_o")}

            # ================= P1: projections (piece-wise) =================
            def p1_load(ac, nsplit):
                x_t = xc_pool.tile([128, KC, 512], bf16, tag="xin", name="x_t")
                c_t = xc_pool.tile([128, KC, 512], bf16, tag="cin", name="c_t")
                step = KC // nsplit
                for si in range(nsplit):
                    ks = slice(si * step, (si + 1) * step)
                    nc.sync.dma_start(
                        out=c_t[:, ks, :],
                        in_=cT.ap().rearrange("(k p) a -> p k a", p=128)[:, ks, ac * 512:(ac + 1) * 512],
                    )
                    nc.sync.dma_start(
                        out=x_t[:, ks, :],
                        in_=xT.ap().rearrange("(k p) a -> p k a", p=128)[:, ks, ac * 512:(ac + 1) * 512],
                    )
                return x_t, c_t

            def p1_pieces(ac, x_t, c_t):
                # 10 compute pieces per chunk: qk chain, cqk chain, 4 v subs, 4 cv subs
                def proj(src_t, w_sb, dst):
                    p_ps = ps_sim.tile([128, 512], fp32, tag="sim", name="p_ps")
                    for kc in range(KC):
                        nc.tensor.matmul(p_ps, w_sb[:, kc, :], src_t[:, kc, :],
                                         start=(kc == 0), stop=(kc == KC - 1))
                    nc.vector.tensor_copy(dst[:, ac * 512:(ac + 1) * 512], p_ps)

                def vnat(src_t, w_sb, dsts, sub):
                    vp = ps_sim.tile([128, 128], fp32, tag="sim", name="vp")
                    for kc in range(KC):
                        nc.tensor.matmul(vp, src_t[:, kc, sub * 128:(sub + 1) * 128],
                                         w_sb[:, kc, :],
                                         start=(kc == 0), stop=(kc == KC - 1))
                    blk = ac * 4 + sub
                    nc.vector.tensor_copy(dsts[0][:, blk, 0:64], vp[:, 0:64])
                    nc.vector.tensor_copy(dsts[1][:, blk, 0:64], vp[:, 64:128])

                yield lambda: proj(c_t, wcqk_sb, cqkT_sb)
                yield lambda: proj(x_t, wqk_sb, qkT_sb)
                for sub in range(4):
                    yield lambda s=sub: vnat(x_t, wv_sb, v_nat, s)
                for sub in range(4):
                    yield lambda s=sub: vnat(c_t, wcv_sb, cv_nat, s)

            fillers = []

            def drain_fillers(k):
                for _ in range(k):
                    if fillers:
                        fillers.pop(0)()

            # ================= attention window =================
            def emit_window(b, layout, jw, fill_rate=1):
                # layout 0 (ctx dir): stat=qk(i), mov=cqk(j), vals=v
                # layout 1 (out dir): stat=cqk(j), mov=qk(i), vals=cv
                if layout == 0:
                    statT, movT, vals, stg_t = qkT_sb, cqkT_sb, v_nat, stg[("c", b)]
                else:
                    statT, movT, vals, stg_t = cqkT_sb, qkT_sb, cv_nat, stg[("o", b)]
                base = b * N
                m0 = base + jw * 512
                pv = [ps_pv.tile([128, 260], fp32, name=f"pv{h}", tag=f"pv{h}") for h in range(2)]

                def emit_pv(it, e):
                    for h in range(2):
                        for jt in range(4):
                            nc.tensor.matmul(
                                pv[h][:, jt * 65:jt * 65 + 65],
                                e[:, h * 512 + jt * 128:h * 512 + (jt + 1) * 128],
                                vals[h][:, b * NT + it, :],
                                start=(it == 0 and jt == 0), stop=(it == NT - 1),
                                skip_group_check=True,
                            )

                # software pipeline: PV steps lag one iteration behind exp so
                # the next window's sim/exp never block on the pv bank release
                pend = []
                for it in range(NT):
                    a0 = base + it * 128
                    s = ps_sim.tile([128, 1024], fp32, tag="sim", name="s")
                    nc.tensor.matmul(s[:, 0:512], statT[0:64, a0:a0 + 128],
                                     movT[0:64, m0:m0 + 512], start=True, stop=True)
                    nc.tensor.matmul(s[:, 512:1024], statT[64:128, a0:a0 + 128],
                                     movT[64:128, m0:m0 + 512], start=True, stop=True,
                                     tile_position=(64, 0))
                    e = e_pool.tile([128, 1024], bf16, tag="e", name="e")
                    nc.scalar.activation(e, s, EXP, scale=SCALE)
                    pend.append((it, e))
                    if len(pend) > 2:
                        emit_pv(*pend.pop(0))
                    drain_fillers(fill_rate)
                for pe_it in pend:
                    emit_pv(*pe_it)
                # normalize: per-token (partition) reciprocal of the ones-column
                for h in range(2):
                    rec = rec_pool.tile([128, 4], fp32, tag="rec", name="rec")
                    nc.vector.reciprocal(
                        rec, pv[h].rearrange("p (j c) -> p j c", c=65)[:, :, 64])
                    for jt in range(4):
                        nc.vector.tensor_scalar_mul(
                            stg_t[:, jw * 4 + jt, h, :],
                            pv[h][:, jt * 65:jt * 65 + 64],
                            rec[:, jt:jt + 1],
                        )

            def emit_bounce(dname, b, jw):
                # bounce this window's 4 token tiles into its half cc buffer;
                # tile tt belongs to core tt % 8, half tt // 8
                stg_t = stg[(dname, b)]
                cc_in = cc_bufs[(dname, b, jw // 2, "in")]
                r0 = (jw * 4) % 8
                nc.sync.dma_start(
                    out=cc_in.ap().rearrange("(t p) hd -> p t hd", p=128)[:, r0:r0 + 4, :],
                    in_=stg_t.rearrange("p t h d -> p t (h d)")[:, jw * 4:(jw + 1) * 4, :],
                )

            def a2a(cin, cout):
                if use_cc:
                    nc.gpsimd.collective_compute(
                        "AllToAll", mybir.AluOpType.bypass,
                        replica_groups=[list(range(N_CORES))],
                        ins=[cin.ap().opt()], outs=[cout.ap().opt()],
                    )
                else:
                    nc.gpsimd.dma_start(out=cout.ap(), in_=cin.ap())

            # post-A2A: [tok, d] received blocks -> [d, tok] full tensor
            def emit_post_t(dname, b):
                for piece in post_t_pieces(dname, b):
                    piece()

            ld_tiles = {}

            def emit_ld(dname, b, on_act=False):
                # cc_out loads emitted right after the h2 collective; the Pool
                # queue keeps them off hot sequencers mid-kernel, the ACT
                # queue (idle by then) is faster for the final exchange
                eng = nc.scalar if on_act else nc.gpsimd
                ld = ld_pool.tile([128, 2, N_CORES, 128], bf16, tag="ld", name="ld")
                ld_tiles[(dname, b)] = ld
                for hh in range(2):
                    ccv = cc_bufs[(dname, b, hh, "out")].ap().rearrange(
                        "(r p) d -> p r d", p=128)
                    eng.dma_start(out=ld[:, hh, :, :], in_=ccv)

            def post_t_pieces(dname, b):
                ld = ld_tiles[(dname, b)]
                for hh in range(2):
                    for r0 in range(0, N_CORES, 2):
                        def piece(hh=hh, r0=r0, ld=ld):
                            for r in (r0, r0 + 1):
                                tp = ps_sim.tile([128, 128], bf16, tag="sim", name="tp")
                                nc.tensor.transpose(tp, ld[:, hh, r, :], ident)
                                nc.vector.tensor_copy(
                                    full[dname][:, r, b * 256 + hh * 128:b * 256 + (hh + 1) * 128], tp)
                        yield piece

            def stage3_pieces(dname, b, at2s=None):
                w_sb = wcout_sb if dname == "c" else wout_sb
                dst = ctx_sl if dname == "c" else out_sl
                for at2 in (at2s if at2s is not None else (2 * b, 2 * b + 1)):
                    ps3 = ps_sim.tile([128, 1024], fp32, tag="sim", name="ps3")
                    for kc0 in range(0, KC, 2):
                        def piece(at2=at2, kc0=kc0, ps3=ps3):
                            for kc in (kc0, kc0 + 1):
                                for g in range(2):
                                    nc.tensor.matmul(
                                        ps3[:, g * 512:(g + 1) * 512],
                                        full[dname][:, kc, at2 * 128:(at2 + 1) * 128],
                                        w_sb[:, kc, g * 512:(g + 1) * 512],
                                        start=(kc == 0), stop=(kc == KC - 1),
                                        skip_group_check=True,
                                    )
                        yield piece

                    def fin(at2=at2, ps3=ps3):
                        o_sb = osb_pool.tile([128, 1024], fp32, tag="o", name="o_sb")
                        nc.vector.tensor_copy(o_sb, ps3)
                        nc.scalar.dma_start(out=dst.ap()[at2 * 128:(at2 + 1) * 128, :], in_=o_sb)
                    yield fin

            def emit_stage3(dname, b, at2s=None):
                w_sb = wcout_sb if dname == "c" else wout_sb
                dst = ctx_sl if dname == "c" else out_sl
                for at2 in (at2s if at2s is not None else (2 * b, 2 * b + 1)):
                    ps3 = ps_sim.tile([128, 1024], fp32, tag="sim", name="ps3")
                    for kc in range(KC):
                        for g in range(2):
                            nc.tensor.matmul(
                                ps3[:, g * 512:(g + 1) * 512],
                                full[dname][:, kc, at2 * 128:(at2 + 1) * 128],
                                w_sb[:, kc, g * 512:(g + 1) * 512],
                                start=(kc == 0), stop=(kc == KC - 1),
                                skip_group_check=True,
                            )
                    o_sb = osb_pool.tile([128, 1024], fp32, tag="o", name="o_sb")
                    nc.vector.tensor_copy(o_sb, ps3)
                    nc.scalar.dma_start(out=dst.ap()[at2 * 128:(at2 + 1) * 128, :], in_=o_sb)

            # ================= schedule =================
            # chunk 0 emitted whole (first window depends on it); rest queued
            x_t, c_t = p1_load(0, 2)
            load_w(wv, wv_sb)
            load_w(wcv, wcv_sb)
            for piece in p1_pieces(0, x_t, c_t):
                piece()
            for ac in (1, 2, 3):
                x_t, c_t = p1_load(ac, 1)
                fillers.extend(p1_pieces(ac, x_t, c_t))

            def phase_a2a(dname, b, jw):
                if jw in (1, 3):
                    hh = jw // 2
                    a2a(cc_bufs[(dname, b, hh, "in")], cc_bufs[(dname, b, hh, "out")])
                if jw == 3:
                    emit_ld(dname, b, on_act=(dname, b) == ("o", 1))

            for jw in range(4):               # L1(b0); P1 pieces as filler
                if jw == 2:
                    x_t, c_t = p1_load(4, 1)
                    fillers.extend(p1_pieces(4, x_t, c_t))
                emit_window(0, 0, jw, fill_rate=2 if jw == 0 else 1)
                emit_bounce("c", 0, jw)
                phase_a2a("c", 0, jw)

            for iw in range(4):               # L2(b0); rest of P1(b1) as filler
                if iw in (0, 2):
                    ac = 5 + iw // 2
                    x_t, c_t = p1_load(ac, 1)
                    fillers.extend(p1_pieces(ac, x_t, c_t))
                if iw == 3:
                    x_t, c_t = p1_load(7, 1)
                    fillers.extend(p1_pieces(7, x_t, c_t))
                emit_window(0, 1, iw, fill_rate=1)
                emit_bounce("o", 0, iw)
                phase_a2a("o", 0, iw)
                if iw == 0:
                    nc.sync.dma_start(out=wcout_sb, in_=wcout.ap().rearrange("(k p) g -> p k g", p=128))
                if iw == 1:
                    nc.sync.dma_start(out=wout_sb, in_=wout.ap().rearrange("(k p) g -> p k g", p=128))

            for jw in range(4):               # L1(b1); c0 tail work as filler
                if jw == 0:
                    fillers.extend(post_t_pieces("c", 0))
                    fillers.extend(stage3_pieces("c", 0))
                emit_window(1, 0, jw, fill_rate=1)
                emit_bounce("c", 1, jw)
                phase_a2a("c", 1, jw)

            for iw in range(4):               # L2(b1); o0 tail work as filler
                if iw == 0:
                    fillers.extend(post_t_pieces("o", 0))
                    fillers.extend(stage3_pieces("o", 0))
                emit_window(1, 1, iw, fill_rate=1)
                emit_bounce("o", 1, iw)
                phase_a2a("o", 1, iw)

            # drain leftover fillers; c1 tail work hides the o1 collective
            drain_fillers(len(fillers))
            emit_post_t("c", 1)
            emit_stage3("c", 1)
            o1_pieces = list(post_t_pieces("o", 1))
            for piece in o1_pieces[:8]:      # half hh=0
                piece()
            s3o1 = list(stage3_pieces("o", 1, at2s=(2,)))
            for k, piece in enumerate(o1_pieces[8:]):
                piece()
                if k < len(s3o1):
                    s3o1[k]()
            for piece in s3o1[len(o1_pieces) - 8:]:
                piece()
            emit_stage3("o", 1, at2s=(3,))
    nc.compile()
    return nc


_NC_CACHE = {}


def _get_nc():
    if "nc" not in _NC_CACHE:
        _NC_CACHE["nc"] = build_nc()
    return _NC_CACHE["nc"]


def _run(in_maps, trace=False):
    from concourse.bass_utils import run_bass_kernel_spmd
    nc = _get_nc()
    return run_bass_kernel_spmd(nc, in_maps, core_ids=list(range(N_CORES)), trace=trace)


def prepare_in_maps(x, context, w_qk, w_v, w_cqk, w_cv, w_out, w_cout):
    x = np.asarray(x, dtype=np.float32)
    context = np.asarray(context, dtype=np.float32)
    xT = np.ascontiguousarray(x.reshape(A, D).T).astype(BF16)
    cT = np.ascontiguousarray(context.reshape(A, D).T).astype(BF16)
    wout_b = np.asarray(w_out, np.float32).astype(BF16)
    wcout_b = np.asarray(w_cout, np.float32).astype(BF16)
    in_maps = []
    for c in range(N_CORES):
        cs = slice(c * HD, (c + 1) * HD)
        in_maps.append({
            "xT": xT,
            "cT": cT,
            "wqk": np.ascontiguousarray(np.asarray(w_qk, np.float32)[:, cs]).astype(BF16),
            "wv": np.ascontiguousarray(np.asarray(w_v, np.float32)[:, cs]).astype(BF16),
            "wcqk": np.ascontiguousarray(np.asarray(w_cqk, np.float32)[:, cs]).astype(BF16),
            "wcv": np.ascontiguousarray(np.asarray(w_cv, np.float32)[:, cs]).astype(BF16),
            "wout": wout_b,
            "wcout": wcout_b,
        })
    return in_maps


def assemble(results):
    # core c owns token tiles {c, c + 8} per batch (out_sl rows are
    # (batch, half, 128)): tile c covers tokens [c*128, (c+1)*128) and tile
    # c+8 covers [1024 + c*128, 1024 + (c+1)*128)
    out = np.empty((B, N, D), np.float32)
    ctx = np.empty((B, N, D), np.float32)
    for c in range(N_CORES):
        o = results[c]["out_sl"].reshape(B, 2, 128, D)
        k = results[c]["ctx_sl"].reshape(B, 2, 128, D)
        for hh in range(2):
            sl = slice(hh * 1024 + c * 128, hh * 1024 + (c + 1) * 128)
            out[:, sl, :] = o[:, hh]
            ctx[:, sl, :] = k[:, hh]
    return out, ctx


def kernel(x, context, w_qk, w_v, w_cqk, w_cv, w_out, w_cout):
    in_maps = prepare_in_maps(x, context, w_qk, w_v, w_cqk, w_cv, w_out, w_cout)
    res = _run(in_maps)
    return assemble(res.results)
